# revision 36
# baseline (speedup 1.0000x reference)
"""Gaussian duration-attention upsampler on 8 Trainium2 NeuronCores.

out[b,t,:] = (sum_i w[b,i,t] * emb[b,i,:]) / (sum_i w[b,i,t] + eps) + PE[t,:]
  with w[b,i,t] = exp(-(t - c[b,i])^2 / ranges[b,i]^2), c = cumsum(dur) - dur/2.

Strategy:
  - Data-parallel over batch: 32 batches -> 4 per core on 8 cores (SPMD, no
    collectives).
  - The Gaussians are narrow (reach <= sqrt(30)*4.5 ~ 25 frames), so W is
    banded: for each 512-frame output chunk only a contiguous window of
    <= 128 tokens matters (max span on this data: ~80). The host picks the
    window starts from c (cheap O(B*T_in) preprocessing) and gathers the
    embedding rows per window.
  - Per (batch, chunk): ScalarE builds W = exp(-a*(t-c)^2) for the 128-token
    window in two activation passes (Square with per-partition bias -c, then
    Exp with per-partition scale -a); TensorE computes [W^T] @ [E|1] in bf16
    per 128-frame sub-chunk (the ones column yields the normalizer s in
    PSUM column 256); VectorE fuses normalize + positional-encoding add with
    one scalar_tensor_tensor per sub-chunk: out = (U * 1/(s+eps)) + PE.
  - Output frames are permuted (t = 512j + 4q + sub on PSUM partition q) so
    each partition's staged row is 1024 contiguous DRAM elements -> clean
    output DMA descriptors.
"""

import numpy as np
import ml_dtypes

import concourse.bacc as bacc
import concourse.bass as bass
import concourse.mybir as mybir
import concourse.tile as tile
from concourse.bass_utils import run_bass_kernel_spmd

BF16 = ml_dtypes.bfloat16

B, T_IN, D, T_OUT = 32, 512, 256, 4096
EPS = 1e-6
N_CORES = 8
BL = B // N_CORES          # batches per core
NJ = T_OUT // 512          # 512-frame output chunks per batch
CW = 512                   # chunk width (frames)
KW = 128                   # window tokens per k-chunk
TH = 30.0                  # exp(-30) ~ 1e-13: banding threshold

F32 = mybir.dt.float32
BF = mybir.dt.bfloat16

_CACHE = {}


def _pe_table():
    pos = np.arange(T_OUT, dtype=np.float32)[:, None]
    div = np.exp(np.arange(0, D, 2, dtype=np.float32) * (-np.log(10000.0) / D))
    pe = np.zeros((T_OUT, D), np.float32)
    pe[:, 0::2] = np.sin(pos * div)
    pe[:, 1::2] = np.cos(pos * div)
    return pe


N_ACT_SUBS = 0   # sub-chunks normalized on ScalarE (rest: DVE stt);
                 # keep 0: an ACT reader of PSUM couples PSUM recycling to the
                 # ScalarE queue (which carries multi-us W-gen batches)
PAIR = 4         # windows sharing one W-gen batch (Exp runs at FD=PAIR*512)
USE_POW = False  # pow is not a valid tensor_scalar ALU op on trn2


def _build(nkc):
    """Build + schedule the SPMD bass graph for nkc 128-token k-chunks."""
    nc = bacc.Bacc(
        "TRN2",
        target_bir_lowering=False,
        debug=False,
        enable_asserts=False,
        num_devices=N_CORES,
    )
    eg_d = nc.dram_tensor("eg", (BL, 128, nkc, NJ, 257), BF, kind="ExternalInput")
    sqa_d = nc.dram_tensor("sqa", (128, BL, nkc, NJ), F32, kind="ExternalInput")
    nsqac_d = nc.dram_tensor("nsqac", (128, BL, nkc, NJ), F32, kind="ExternalInput")
    iota_d = nc.dram_tensor("iota", (128, CW), F32, kind="ExternalInput")
    pe_d = nc.dram_tensor("pe", (128, NJ * 4 * D), BF, kind="ExternalInput")
    out_d = nc.dram_tensor("out", (BL, T_OUT, D), BF, kind="ExternalOutput")
    # frame t = 512j + 4q + sub lives on partition q, free offset sub*D + d
    out_v = out_d[:].rearrange("b (j q s) d -> b j q (s d)", j=NJ, q=128, s=4)

    Sq = mybir.ActivationFunctionType.Square
    Ex = mybir.ActivationFunctionType.Exp
    Cp = mybir.ActivationFunctionType.Copy
    ADD = mybir.AluOpType.add
    MUL = mybir.AluOpType.mult

    with tile.TileContext(nc) as tc:
        with (
            tc.tile_pool(name="const", bufs=1) as cp,
            tc.tile_pool(name="eg", bufs=BL) as egp,
            tc.tile_pool(name="sq", bufs=3) as sqp,
            tc.tile_pool(name="w", bufs=3) as wp,
            tc.tile_pool(name="ps", bufs=2, space="PSUM") as psp,
            tc.tile_pool(name="rr", bufs=12) as rp,
            tc.tile_pool(name="un", bufs=6) as unp,
            tc.tile_pool(name="ob", bufs=6) as obp,
        ):
            # dummy activation with no DMA deps: forces the ACT table load to
            # the head of the Scalar queue, overlapping it with input DMAs
            dmy = cp.tile([128, 8], F32)
            nc.gpsimd.memset(dmy[:], 0.0)
            dmy2 = cp.tile([128, 8], F32)
            nc.scalar.activation(dmy2[:], dmy[:], mybir.ActivationFunctionType.Square)
            nc.scalar.activation(dmy2[:], dmy[:], mybir.ActivationFunctionType.Exp)

            sqa_sb = cp.tile([128, BL, nkc, NJ], F32)
            nc.sync.dma_start(sqa_sb[:], sqa_d[:])
            nsqac_sb = cp.tile([128, BL, nkc, NJ], F32)
            nc.sync.dma_start(nsqac_sb[:], nsqac_d[:])
            iota_sb = cp.tile([128, CW], F32)
            nc.sync.dma_start(iota_sb[:], iota_d[:])
            eg_sbs = [
                egp.tile([128, nkc, NJ, 257], BF, name=f"egt{b}", tag=f"eg{b}")
                for b in range(BL)
            ]
            nc.sync.dma_start(eg_sbs[0][:, :, 0:1, :], eg_d[0][:, :, 0:1, :])
            nc.sync.dma_start(eg_sbs[0][:, :, 1:, :], eg_d[0][:, :, 1:, :])
            pe_sb = cp.tile([128, NJ * 4 * D], BF)
            half = NJ * 2 * D
            nc.sync.dma_start(pe_sb[:, :half], pe_d[:, :half])
            nc.sync.dma_start(pe_sb[:, half:], pe_d[:, half:])
            for b in range(1, BL):
                nc.sync.dma_start(eg_sbs[b][:], eg_d[b])

            PW = PAIR * nkc * CW     # W columns per quad-group
            quads = [
                (b, list(range(j0, min(j0 + PAIR, NJ))))
                for b in range(BL)
                for j0 in range(0, NJ, PAIR)
            ]

            def wgen_sq(q, split=False):
                """sq = (sqrt(a)*t' - sqrt(a)*c')^2 per window (per-partition
                scale+bias on ScalarE). When split, also emit the per-window
                Exp right away (shortens the startup critical path)."""
                b, js = q
                sq_b = sqp.tile([128, PW], F32, tag="sq")
                w_b = wp.tile([128, PW], BF, tag="w")
                for ji, j in enumerate(js):
                    for kc in range(nkc):
                        o = (ji * nkc + kc) * CW
                        nc.scalar.activation(
                            sq_b[:, o : o + CW], iota_sb[:], Sq,
                            bias=nsqac_sb[:, b, kc, j : j + 1],
                            scale=sqa_sb[:, b, kc, j : j + 1],
                        )
                        if split:
                            nc.scalar.activation(
                                w_b[:, o : o + CW], sq_b[:, o : o + CW],
                                Ex, scale=-1.0,
                            )
                return sq_b, w_b, split

            def wgen_exp(wg):
                sq_b, w_b, split = wg
                if not split:
                    nc.scalar.activation(w_b[:], sq_b[:], Ex, scale=-1.0)
                return w_b

            gidx = 0
            w_next = wgen_exp(wgen_sq(quads[0], split=True))
            for qi, (b, js) in enumerate(quads):
                w_b = w_next
                if qi + 1 < len(quads):
                    # next quad's Sq ops go to the ScalarE queue ahead of this
                    # quad's norm-copies (which wait on matmuls); its Exp is
                    # emitted after them (postproc loop below)
                    wg_next = wgen_sq(quads[qi + 1])
                else:
                    wg_next = None
                if True:
                    for ji, j in enumerate(js):
                        ps = psp.tile([128, 4, 512], F32)
                        for sub in range(4):
                            for kc in range(nkc):
                                o = (ji * nkc + kc) * CW + sub * 128
                                nc.tensor.matmul(
                                    ps[:, sub, 0:257],
                                    w_b[:, o : o + 128],
                                    eg_sbs[b][:, kc, j, :],
                                    start=(kc == 0),
                                    stop=(kc == nkc - 1),
                                )
                        s4 = rp.tile([128, 4], F32)
                        nc.vector.tensor_scalar(s4[:], ps[:, :, 256], EPS, None, ADD)
                        r4 = rp.tile([128, 4], F32)
                        nc.vector.reciprocal(r4[:], s4[:])
                        ob = obp.tile([128, 4 * D], BF)
                        n_act = N_ACT_SUBS
                        gidx += 1
                        lo = 4 - n_act
                        if n_act:
                            for sub in range(lo, 4):
                                nc.scalar.activation(
                                    ob[:, sub * D : (sub + 1) * D],
                                    ps[:, sub, 0:D],
                                    Cp,
                                    scale=r4[:, sub : sub + 1],
                                )
                        for sub in range(lo):
                            g = j * 4 + sub
                            nc.vector.scalar_tensor_tensor(
                                ob[:, sub * D : (sub + 1) * D],
                                ps[:, sub, 0:D],
                                r4[:, sub : sub + 1],
                                pe_sb[:, g * D : (g + 1) * D],
                                MUL,
                                ADD,
                            )
                        if n_act:
                            # PE-add for the ACT-normalized subs rides the DMA
                            # engines (CCE accumulate), not VectorE
                            nc.gpsimd.dma_start(
                                ob[:, lo * D :],
                                pe_sb[:, (j * 4 + lo) * D : (j * 4 + 4) * D],
                                accum_op=ADD,
                            )
                        nc.gpsimd.dma_start(out_v[b, j], ob[:])
                if wg_next is not None:
                    w_next = wgen_exp(wg_next)

    nc.compile()
    return nc


def kernel(embeddings, durations, ranges, t_out):
    assert int(t_out) == T_OUT
    emb = np.asarray(embeddings, dtype=np.float32)
    dur = np.asarray(durations, dtype=np.float32)[:, :, 0]
    rng = np.asarray(ranges, dtype=np.float32)[:, :, 0]

    # ---- host preprocessing: O(B*T_in) scalars + window selection ----
    c = np.cumsum(dur, axis=1, dtype=np.float32) - 0.5 * dur   # (B, T_IN)
    a = rng.astype(np.float32) ** -2
    reach = np.sqrt(TH) / np.sqrt(a)

    # window starts: tokens whose gaussian reaches into chunk j
    starts = np.zeros((B, NJ), np.int32)
    span_max = 1
    for b in range(B):
        lo_r, hi_r = c[b] - reach[b], c[b] + reach[b]
        for j in range(NJ):
            qual = np.nonzero((lo_r <= CW * j + CW - 1) & (hi_r >= CW * j))[0]
            if len(qual):
                span_max = max(span_max, int(np.ceil((qual[-1] - qual[0] + 1) / KW)))
                starts[b, j] = qual[0]
            else:
                starts[b, j] = 0
    nkc = span_max
    starts = np.minimum(starts, T_IN - KW * nkc)
    # coverage assert (windows are contiguous token ranges)
    for b in range(B):
        lo_r, hi_r = c[b] - reach[b], c[b] + reach[b]
        for j in range(NJ):
            qual = np.nonzero((lo_r <= CW * j + CW - 1) & (hi_r >= CW * j))[0]
            if len(qual):
                assert starts[b, j] <= qual[0] and qual[-1] < starts[b, j] + KW * nkc

    # gathered per-window tensors
    ea = np.ones((B, T_IN, 257), BF16)
    ea[:, :, :256] = emb.astype(BF16)
    kidx = starts[:, None, :, None] + (
        np.arange(nkc)[None, :, None, None] * KW + np.arange(KW)[None, None, None, :]
    )  # (B, nkc, NJ, KW)
    bidx = np.arange(B)[:, None, None, None]
    eg = ea[bidx, kidx]                       # (B, nkc, NJ, KW, 257)
    eg = eg.transpose(0, 3, 1, 2, 4).copy()   # (B, KW, nkc, NJ, 257)
    cg = c[bidx, kidx]                        # (B, nkc, NJ, KW)
    ag = a[bidx, kidx]
    jgrid = np.arange(NJ, dtype=np.float32)[None, None, :, None] * CW
    sqa_g = np.sqrt(ag)
    sqa = sqa_g.transpose(3, 0, 1, 2).astype(np.float32).copy()          # (KW,B,nkc,NJ)
    nsqac = (sqa_g * (jgrid - cg)).transpose(3, 0, 1, 2).astype(np.float32).copy()

    # constants: permuted iota (col f of W is frame t' = 4*(f%128) + f//128)
    f = np.arange(CW)
    tperm = (4 * (f % 128) + f // 128).astype(np.float32)
    iota = np.broadcast_to(tperm, (128, CW)).copy()
    # PE in the same permuted layout: tile[q, j*1024 + sub*256 + d] = PE[512j+4q+sub, d]
    pe = _pe_table().reshape(NJ, 128, 4, D).transpose(1, 0, 2, 3).reshape(128, -1)
    pe = pe.astype(BF16)

    key = nkc
    if key not in _CACHE:
        _CACHE[key] = _build(nkc)
    nc = _CACHE[key]

    in_maps = []
    for i in range(N_CORES):
        bs = slice(i * BL, (i + 1) * BL)
        in_maps.append({
            "eg": np.ascontiguousarray(eg[bs]),
            "sqa": np.ascontiguousarray(sqa[:, bs]),
            "nsqac": np.ascontiguousarray(nsqac[:, bs]),
            "iota": iota,
            "pe": pe,
        })

    res = run_bass_kernel_spmd(nc, in_maps, core_ids=list(range(N_CORES)))
    out = np.concatenate([r["out"] for r in res.results], axis=0)
    return out.astype(np.float32)


# revision 38
# speedup vs baseline: 1.0641x; 1.0641x over previous
"""Gaussian duration-attention upsampler on 8 Trainium2 NeuronCores.

out[b,t,:] = (sum_i w[b,i,t] * emb[b,i,:]) / (sum_i w[b,i,t] + eps) + PE[t,:]
  with w[b,i,t] = exp(-(t - c[b,i])^2 / ranges[b,i]^2), c = cumsum(dur) - dur/2.

Strategy:
  - Data-parallel over batch: 32 batches -> 4 per core on 8 cores (SPMD, no
    collectives).
  - The Gaussians are narrow (reach <= sqrt(30)*4.5 ~ 25 frames), so W is
    banded: for each 512-frame output chunk only a contiguous window of
    <= 128 tokens matters (max span on this data: ~80). The host picks the
    window starts from c (cheap O(B*T_in) preprocessing) and gathers the
    embedding rows per window.
  - Per (batch, chunk): ScalarE builds W = exp(-a*(t-c)^2) for the 128-token
    window in two activation passes (Square with per-partition bias -c, then
    Exp with per-partition scale -a); TensorE computes [W^T] @ [E|1] in bf16
    per 128-frame sub-chunk (the ones column yields the normalizer s in
    PSUM column 256); VectorE fuses normalize + positional-encoding add with
    one scalar_tensor_tensor per sub-chunk: out = (U * 1/(s+eps)) + PE.
  - Output frames are permuted (t = 512j + 4q + sub on PSUM partition q) so
    each partition's staged row is 1024 contiguous DRAM elements -> clean
    output DMA descriptors.
"""

import numpy as np
import ml_dtypes

import concourse.bacc as bacc
import concourse.bass as bass
import concourse.mybir as mybir
import concourse.tile as tile
from concourse.bass_utils import run_bass_kernel_spmd

BF16 = ml_dtypes.bfloat16

B, T_IN, D, T_OUT = 32, 512, 256, 4096
EPS = 1e-6
N_CORES = 8
BL = B // N_CORES          # batches per core
NJ = T_OUT // 512          # 512-frame output chunks per batch
CW = 512                   # chunk width (frames)
KW = 128                   # window tokens per k-chunk
TH = 30.0                  # exp(-30) ~ 1e-13: banding threshold

F32 = mybir.dt.float32
BF = mybir.dt.bfloat16

_CACHE = {}


def _pe_table():
    pos = np.arange(T_OUT, dtype=np.float32)[:, None]
    div = np.exp(np.arange(0, D, 2, dtype=np.float32) * (-np.log(10000.0) / D))
    pe = np.zeros((T_OUT, D), np.float32)
    pe[:, 0::2] = np.sin(pos * div)
    pe[:, 1::2] = np.cos(pos * div)
    return pe


N_ACT_SUBS = 0   # sub-chunks normalized on ScalarE (rest: DVE stt);
                 # keep 0: an ACT reader of PSUM couples PSUM recycling to the
                 # ScalarE queue (which carries multi-us W-gen batches)
PAIR = 4         # windows sharing one W-gen batch (Exp runs at FD=PAIR*512)
USE_POW = False  # pow is not a valid tensor_scalar ALU op on trn2


def _build(nkc):
    """Build + schedule the SPMD bass graph for nkc 128-token k-chunks."""
    nc = bacc.Bacc(
        "TRN2",
        target_bir_lowering=False,
        debug=False,
        enable_asserts=False,
        num_devices=N_CORES,
    )
    eg_d = nc.dram_tensor("eg", (BL, 128, nkc, NJ, 257), BF, kind="ExternalInput")
    sqa_d = nc.dram_tensor("sqa", (128, BL, nkc, NJ), F32, kind="ExternalInput")
    nsqac_d = nc.dram_tensor("nsqac", (128, BL, nkc, NJ), F32, kind="ExternalInput")
    iota_d = nc.dram_tensor("iota", (128, CW), F32, kind="ExternalInput")
    pe_d = nc.dram_tensor("pe", (128, NJ * 4 * D), BF, kind="ExternalInput")
    out_d = nc.dram_tensor("out", (BL, T_OUT, D), BF, kind="ExternalOutput")
    # frame t = 512j + 4q + sub lives on partition q, free offset sub*D + d
    out_v = out_d[:].rearrange("b (j q s) d -> b j q (s d)", j=NJ, q=128, s=4)

    Sq = mybir.ActivationFunctionType.Square
    Ex = mybir.ActivationFunctionType.Exp
    Cp = mybir.ActivationFunctionType.Copy
    ADD = mybir.AluOpType.add
    MUL = mybir.AluOpType.mult

    with tile.TileContext(nc) as tc:
        with (
            tc.tile_pool(name="const", bufs=1) as cp,
            tc.tile_pool(name="eg", bufs=BL) as egp,
            tc.tile_pool(name="sq", bufs=3) as sqp,
            tc.tile_pool(name="w", bufs=3) as wp,
            tc.tile_pool(name="ps", bufs=2, space="PSUM") as psp,
            tc.tile_pool(name="rr", bufs=12) as rp,
            tc.tile_pool(name="un", bufs=6) as unp,
            tc.tile_pool(name="ob", bufs=6) as obp,
        ):
            # dummy activation with no DMA deps: forces the ACT table load to
            # the head of the Scalar queue, overlapping it with input DMAs
            dmy = cp.tile([128, 8], F32)
            nc.gpsimd.memset(dmy[:], 0.0)
            dmy2 = cp.tile([128, 8], F32)
            nc.scalar.activation(dmy2[:], dmy[:], mybir.ActivationFunctionType.Square)
            nc.scalar.activation(dmy2[:], dmy[:], mybir.ActivationFunctionType.Exp)

            sqa_sb = cp.tile([128, BL, nkc, NJ], F32)
            nc.sync.dma_start(sqa_sb[:], sqa_d[:])
            nsqac_sb = cp.tile([128, BL, nkc, NJ], F32)
            nc.sync.dma_start(nsqac_sb[:], nsqac_d[:])
            iota_sb = cp.tile([128, CW], F32)
            nc.sync.dma_start(iota_sb[:], iota_d[:])
            eg_sbs = [
                egp.tile([128, nkc, NJ, 257], BF, name=f"egt{b}", tag=f"eg{b}")
                for b in range(BL)
            ]
            nc.sync.dma_start(eg_sbs[0][:, :, 0:1, :], eg_d[0][:, :, 0:1, :])
            nc.sync.dma_start(eg_sbs[0][:, :, 1:, :], eg_d[0][:, :, 1:, :])
            pe_sb = cp.tile([128, NJ * 4 * D], BF)
            half = NJ * 2 * D
            nc.sync.dma_start(pe_sb[:, :half], pe_d[:, :half])
            nc.sync.dma_start(pe_sb[:, half:], pe_d[:, half:])
            for b in range(1, BL):
                nc.sync.dma_start(eg_sbs[b][:], eg_d[b])

            PW = PAIR * nkc * CW     # W columns per quad-group
            quads = [
                (b, list(range(j0, min(j0 + PAIR, NJ))))
                for b in range(BL)
                for j0 in range(0, NJ, PAIR)
            ]

            def wgen_sq(q, split=False):
                """sq = (sqrt(a)*t' - sqrt(a)*c')^2 per window (per-partition
                scale+bias on ScalarE). When split, also emit the per-window
                Exp right away (shortens the startup critical path)."""
                b, js = q
                sq_b = sqp.tile([128, PW], F32, tag="sq")
                w_b = wp.tile([128, PW], BF, tag="w")
                for ji, j in enumerate(js):
                    for kc in range(nkc):
                        o = (ji * nkc + kc) * CW
                        nc.scalar.activation(
                            sq_b[:, o : o + CW], iota_sb[:], Sq,
                            bias=nsqac_sb[:, b, kc, j : j + 1],
                            scale=sqa_sb[:, b, kc, j : j + 1],
                        )
                        if split:
                            nc.scalar.activation(
                                w_b[:, o : o + CW], sq_b[:, o : o + CW],
                                Ex, scale=-1.0,
                            )
                return sq_b, w_b, split

            def wgen_exp(wg):
                sq_b, w_b, split = wg
                if not split:
                    nc.scalar.activation(w_b[:], sq_b[:], Ex, scale=-1.0)
                return w_b

            gidx = 0
            w_next = wgen_exp(wgen_sq(quads[0], split=True))
            for qi, (b, js) in enumerate(quads):
                w_b = w_next
                if qi + 1 < len(quads):
                    # next quad's Sq ops go to the ScalarE queue ahead of this
                    # quad's norm-copies (which wait on matmuls); its Exp is
                    # emitted after them (postproc loop below)
                    wg_next = wgen_sq(quads[qi + 1])
                else:
                    wg_next = None
                if True:
                    for ji, j in enumerate(js):
                        ps = psp.tile([128, 4, 512], F32)
                        for sub in range(4):
                            for kc in range(nkc):
                                o = (ji * nkc + kc) * CW + sub * 128
                                nc.tensor.matmul(
                                    ps[:, sub, 0:257],
                                    w_b[:, o : o + 128],
                                    eg_sbs[b][:, kc, j, :],
                                    start=(kc == 0),
                                    stop=(kc == nkc - 1),
                                )
                        s4 = rp.tile([128, 4], F32)
                        nc.vector.tensor_scalar(s4[:], ps[:, :, 256], EPS, None, ADD)
                        r4 = rp.tile([128, 4], F32)
                        nc.vector.reciprocal(r4[:], s4[:])
                        ob = obp.tile([128, 4 * D], BF)
                        n_act = N_ACT_SUBS
                        gidx += 1
                        lo = 4 - n_act
                        if n_act:
                            un = unp.tile([128, 2 * D], BF, tag="un")
                            for i, sub in enumerate(range(lo, 4)):
                                nc.scalar.activation(
                                    un[:, i * D : (i + 1) * D],
                                    ps[:, sub, 0:D],
                                    Cp,
                                    scale=r4[:, sub : sub + 1],
                                )
                        for sub in range(lo):
                            g = j * 4 + sub
                            nc.vector.scalar_tensor_tensor(
                                ob[:, sub * D : (sub + 1) * D],
                                ps[:, sub, 0:D],
                                r4[:, sub : sub + 1],
                                pe_sb[:, g * D : (g + 1) * D],
                                MUL,
                                ADD,
                            )
                        if n_act:
                            nc.vector.tensor_add(
                                ob[:, lo * D :],
                                un[:, : n_act * D],
                                pe_sb[:, (j * 4 + lo) * D : (j * 4 + 4) * D],
                            )
                        nc.gpsimd.dma_start(out_v[b, j], ob[:])
                if wg_next is not None:
                    w_next = wgen_exp(wg_next)

    nc.compile()
    return nc


def kernel(embeddings, durations, ranges, t_out):
    assert int(t_out) == T_OUT
    emb = np.asarray(embeddings, dtype=np.float32)
    dur = np.asarray(durations, dtype=np.float32)[:, :, 0]
    rng = np.asarray(ranges, dtype=np.float32)[:, :, 0]

    # ---- host preprocessing: O(B*T_in) scalars + window selection ----
    c = np.cumsum(dur, axis=1, dtype=np.float32) - 0.5 * dur   # (B, T_IN)
    a = rng.astype(np.float32) ** -2
    reach = np.sqrt(TH) / np.sqrt(a)

    # window starts: tokens whose gaussian reaches into chunk j
    starts = np.zeros((B, NJ), np.int32)
    span_max = 1
    for b in range(B):
        lo_r, hi_r = c[b] - reach[b], c[b] + reach[b]
        for j in range(NJ):
            qual = np.nonzero((lo_r <= CW * j + CW - 1) & (hi_r >= CW * j))[0]
            if len(qual):
                span_max = max(span_max, int(np.ceil((qual[-1] - qual[0] + 1) / KW)))
                starts[b, j] = qual[0]
            else:
                starts[b, j] = 0
    nkc = span_max
    starts = np.minimum(starts, T_IN - KW * nkc)
    # coverage assert (windows are contiguous token ranges)
    for b in range(B):
        lo_r, hi_r = c[b] - reach[b], c[b] + reach[b]
        for j in range(NJ):
            qual = np.nonzero((lo_r <= CW * j + CW - 1) & (hi_r >= CW * j))[0]
            if len(qual):
                assert starts[b, j] <= qual[0] and qual[-1] < starts[b, j] + KW * nkc

    # gathered per-window tensors
    ea = np.ones((B, T_IN, 257), BF16)
    ea[:, :, :256] = emb.astype(BF16)
    kidx = starts[:, None, :, None] + (
        np.arange(nkc)[None, :, None, None] * KW + np.arange(KW)[None, None, None, :]
    )  # (B, nkc, NJ, KW)
    bidx = np.arange(B)[:, None, None, None]
    eg = ea[bidx, kidx]                       # (B, nkc, NJ, KW, 257)
    eg = eg.transpose(0, 3, 1, 2, 4).copy()   # (B, KW, nkc, NJ, 257)
    cg = c[bidx, kidx]                        # (B, nkc, NJ, KW)
    ag = a[bidx, kidx]
    jgrid = np.arange(NJ, dtype=np.float32)[None, None, :, None] * CW
    sqa_g = np.sqrt(ag)
    sqa = sqa_g.transpose(3, 0, 1, 2).astype(np.float32).copy()          # (KW,B,nkc,NJ)
    nsqac = (sqa_g * (jgrid - cg)).transpose(3, 0, 1, 2).astype(np.float32).copy()

    # constants: permuted iota (col f of W is frame t' = 4*(f%128) + f//128)
    f = np.arange(CW)
    tperm = (4 * (f % 128) + f // 128).astype(np.float32)
    iota = np.broadcast_to(tperm, (128, CW)).copy()
    # PE in the same permuted layout: tile[q, j*1024 + sub*256 + d] = PE[512j+4q+sub, d]
    pe = _pe_table().reshape(NJ, 128, 4, D).transpose(1, 0, 2, 3).reshape(128, -1)
    pe = pe.astype(BF16)

    key = nkc
    if key not in _CACHE:
        _CACHE[key] = _build(nkc)
    nc = _CACHE[key]

    in_maps = []
    for i in range(N_CORES):
        bs = slice(i * BL, (i + 1) * BL)
        in_maps.append({
            "eg": np.ascontiguousarray(eg[bs]),
            "sqa": np.ascontiguousarray(sqa[:, bs]),
            "nsqac": np.ascontiguousarray(nsqac[:, bs]),
            "iota": iota,
            "pe": pe,
        })

    res = run_bass_kernel_spmd(nc, in_maps, core_ids=list(range(N_CORES)))
    out = np.concatenate([r["out"] for r in res.results], axis=0)
    return out.astype(np.float32)


# revision 43
# speedup vs baseline: 1.0795x; 1.0144x over previous
"""Gaussian duration-attention upsampler on 8 Trainium2 NeuronCores.

out[b,t,:] = (sum_i w[b,i,t] * emb[b,i,:]) / (sum_i w[b,i,t] + eps) + PE[t,:]
  with w[b,i,t] = exp(-(t - c[b,i])^2 / ranges[b,i]^2), c = cumsum(dur) - dur/2.

Strategy:
  - Data-parallel over batch: 32 batches -> 4 per core on 8 cores (SPMD, no
    collectives).
  - The Gaussians are narrow (reach <= sqrt(30)*4.5 ~ 25 frames), so W is
    banded: for each 512-frame output chunk only a contiguous window of
    <= 128 tokens matters (max span on this data: ~80). The host picks the
    window starts from c (cheap O(B*T_in) preprocessing) and gathers the
    embedding rows per window.
  - Per (batch, chunk): ScalarE builds W = exp(-a*(t-c)^2) for the 128-token
    window in two activation passes (Square with per-partition bias -c, then
    Exp with per-partition scale -a); TensorE computes [W^T] @ [E|1] in bf16
    per 128-frame sub-chunk (the ones column yields the normalizer s in
    PSUM column 256); VectorE fuses normalize + positional-encoding add with
    one scalar_tensor_tensor per sub-chunk: out = (U * 1/(s+eps)) + PE.
  - Output frames are permuted (t = 512j + 4q + sub on PSUM partition q) so
    each partition's staged row is 1024 contiguous DRAM elements -> clean
    output DMA descriptors.
"""

import numpy as np
import ml_dtypes

import concourse.bacc as bacc
import concourse.bass as bass
import concourse.mybir as mybir
import concourse.tile as tile
from concourse.bass_utils import run_bass_kernel_spmd

BF16 = ml_dtypes.bfloat16

B, T_IN, D, T_OUT = 32, 512, 256, 4096
EPS = 1e-6
N_CORES = 8
BL = B // N_CORES          # batches per core
NJ = T_OUT // 512          # 512-frame output chunks per batch
CW = 512                   # chunk width (frames)
KW = 128                   # window tokens per k-chunk
TH = 30.0                  # exp(-30) ~ 1e-13: banding threshold

F32 = mybir.dt.float32
BF = mybir.dt.bfloat16

_CACHE = {}


def _pe_table():
    pos = np.arange(T_OUT, dtype=np.float32)[:, None]
    div = np.exp(np.arange(0, D, 2, dtype=np.float32) * (-np.log(10000.0) / D))
    pe = np.zeros((T_OUT, D), np.float32)
    pe[:, 0::2] = np.sin(pos * div)
    pe[:, 1::2] = np.cos(pos * div)
    return pe


N_ACT_SUBS = 0   # sub-chunks normalized on ScalarE (rest: DVE stt);
                 # keep 0: an ACT reader of PSUM couples PSUM recycling to the
                 # ScalarE queue (which carries multi-us W-gen batches)
PAIR = 4         # windows sharing one W-gen batch (Exp runs at FD=PAIR*512)
USE_POW = False  # pow is not a valid tensor_scalar ALU op on trn2


def _build(nkc):
    """Build + schedule the SPMD bass graph for nkc 128-token k-chunks."""
    nc = bacc.Bacc(
        "TRN2",
        target_bir_lowering=False,
        debug=False,
        enable_asserts=False,
        num_devices=N_CORES,
    )
    eg_d = nc.dram_tensor("eg", (BL, 128, nkc, NJ, 257), BF, kind="ExternalInput")
    # params: [sqa (BL*nkc*NJ) | nsqac (BL*nkc*NJ) | iota (CW)] per partition
    NP = BL * nkc * NJ
    par_d = nc.dram_tensor("par", (128, 2 * NP + CW), F32, kind="ExternalInput")
    pe_d = nc.dram_tensor("pe", (128, NJ * 4 * D), BF, kind="ExternalInput")
    out_d = nc.dram_tensor("out", (BL, T_OUT, D), BF, kind="ExternalOutput")
    # frame t = 512j + 4q + sub lives on partition q, free offset sub*D + d
    out_v = out_d[:].rearrange("b (j q s) d -> b j q (s d)", j=NJ, q=128, s=4)

    Sq = mybir.ActivationFunctionType.Square
    Ex = mybir.ActivationFunctionType.Exp
    Cp = mybir.ActivationFunctionType.Copy
    ADD = mybir.AluOpType.add
    MUL = mybir.AluOpType.mult

    with tile.TileContext(nc) as tc:
        with (
            tc.tile_pool(name="const", bufs=1) as cp,
            tc.tile_pool(name="eg", bufs=BL) as egp,
            tc.tile_pool(name="sq", bufs=3) as sqp,
            tc.tile_pool(name="w", bufs=3) as wp,
            tc.tile_pool(name="ps", bufs=2, space="PSUM") as psp,
            tc.tile_pool(name="rr", bufs=12) as rp,
            tc.tile_pool(name="un", bufs=6) as unp,
            tc.tile_pool(name="ob", bufs=6) as obp,
        ):
            # dummy activation with no DMA deps: forces the ACT table load to
            # the head of the Scalar queue, overlapping it with input DMAs
            dmy = cp.tile([128, 8], F32)
            nc.gpsimd.memset(dmy[:], 0.0)
            dmy2 = cp.tile([128, 8], F32)
            nc.scalar.activation(dmy2[:], dmy[:], mybir.ActivationFunctionType.Square)
            nc.scalar.activation(dmy2[:], dmy[:], mybir.ActivationFunctionType.Exp)

            par_sb = cp.tile([128, 2 * NP + CW], F32)
            nc.sync.dma_start(par_sb[:], par_d[:])
            sqa_sb = par_sb[:, 0:NP].rearrange("p (b k j) -> p b k j", b=BL, k=nkc)
            nsqac_sb = par_sb[:, NP : 2 * NP].rearrange(
                "p (b k j) -> p b k j", b=BL, k=nkc
            )
            iota_sb = par_sb[:, 2 * NP :]
            eg_sbs = [
                egp.tile([128, nkc, NJ, 257], BF, name=f"egt{b}", tag=f"eg{b}")
                for b in range(BL)
            ]
            nc.sync.dma_start(eg_sbs[0][:, :, 0:1, :], eg_d[0][:, :, 0:1, :])
            nc.sync.dma_start(eg_sbs[0][:, :, 1:, :], eg_d[0][:, :, 1:, :])
            pe_sb = cp.tile([128, NJ * 4 * D], BF)
            half = NJ * 2 * D
            nc.sync.dma_start(pe_sb[:, :half], pe_d[:, :half])
            nc.sync.dma_start(pe_sb[:, half:], pe_d[:, half:])
            for b in range(1, BL):
                nc.sync.dma_start(eg_sbs[b][:], eg_d[b])

            PW = PAIR * nkc * CW     # W columns per quad-group
            quads = [
                (b, list(range(j0, min(j0 + PAIR, NJ))))
                for b in range(BL)
                for j0 in range(0, NJ, PAIR)
            ]

            def wgen_sq(q, split=False):
                """sq = (sqrt(a)*t' - sqrt(a)*c')^2 per window (per-partition
                scale+bias on ScalarE). When split, also emit the per-window
                Exp right away (shortens the startup critical path)."""
                b, js = q
                sq_b = sqp.tile([128, PW], F32, tag="sq")
                w_b = wp.tile([128, PW], BF, tag="w")
                for ji, j in enumerate(js):
                    for kc in range(nkc):
                        o = (ji * nkc + kc) * CW
                        nc.scalar.activation(
                            sq_b[:, o : o + CW], iota_sb, Sq,
                            bias=nsqac_sb[:, b, kc, j : j + 1],
                            scale=sqa_sb[:, b, kc, j : j + 1],
                        )
                        if split:
                            nc.scalar.activation(
                                w_b[:, o : o + CW], sq_b[:, o : o + CW],
                                Ex, scale=-1.0,
                            )
                return sq_b, w_b, split

            def wgen_exp(wg):
                sq_b, w_b, split = wg
                if not split:
                    nc.scalar.activation(w_b[:], sq_b[:], Ex, scale=-1.0)
                return w_b

            gidx = 0
            w_next = wgen_exp(wgen_sq(quads[0], split=True))
            for qi, (b, js) in enumerate(quads):
                w_b = w_next
                if qi + 1 < len(quads):
                    # next quad's Sq ops go to the ScalarE queue ahead of this
                    # quad's norm-copies (which wait on matmuls); its Exp is
                    # emitted after them (postproc loop below)
                    wg_next = wgen_sq(quads[qi + 1])
                else:
                    wg_next = None
                if True:
                    for ji, j in enumerate(js):
                        ps = psp.tile([128, 4, 512], F32)
                        for sub in range(4):
                            for kc in range(nkc):
                                o = (ji * nkc + kc) * CW + sub * 128
                                nc.tensor.matmul(
                                    ps[:, sub, 0:257],
                                    w_b[:, o : o + 128],
                                    eg_sbs[b][:, kc, j, :],
                                    start=(kc == 0),
                                    stop=(kc == nkc - 1),
                                )
                        s4 = rp.tile([128, 4], F32)
                        nc.vector.tensor_scalar(s4[:], ps[:, :, 256], EPS, None, ADD)
                        r4 = rp.tile([128, 4], F32)
                        nc.vector.reciprocal(r4[:], s4[:])
                        ob = obp.tile([128, 4 * D], BF)
                        n_act = N_ACT_SUBS
                        gidx += 1
                        lo = 4 - n_act
                        if n_act:
                            un = unp.tile([128, 2 * D], BF, tag="un")
                            for i, sub in enumerate(range(lo, 4)):
                                nc.scalar.activation(
                                    un[:, i * D : (i + 1) * D],
                                    ps[:, sub, 0:D],
                                    Cp,
                                    scale=r4[:, sub : sub + 1],
                                )
                        for sub in range(lo):
                            g = j * 4 + sub
                            nc.vector.scalar_tensor_tensor(
                                ob[:, sub * D : (sub + 1) * D],
                                ps[:, sub, 0:D],
                                r4[:, sub : sub + 1],
                                pe_sb[:, g * D : (g + 1) * D],
                                MUL,
                                ADD,
                            )
                        if n_act:
                            nc.vector.tensor_add(
                                ob[:, lo * D :],
                                un[:, : n_act * D],
                                pe_sb[:, (j * 4 + lo) * D : (j * 4 + 4) * D],
                            )
                        nc.gpsimd.dma_start(out_v[b, j], ob[:])
                if wg_next is not None:
                    w_next = wgen_exp(wg_next)

    nc.compile()
    return nc


def kernel(embeddings, durations, ranges, t_out):
    assert int(t_out) == T_OUT
    emb = np.asarray(embeddings, dtype=np.float32)
    dur = np.asarray(durations, dtype=np.float32)[:, :, 0]
    rng = np.asarray(ranges, dtype=np.float32)[:, :, 0]

    # ---- host preprocessing: O(B*T_in) scalars + window selection ----
    c = np.cumsum(dur, axis=1, dtype=np.float32) - 0.5 * dur   # (B, T_IN)
    a = rng.astype(np.float32) ** -2
    reach = np.sqrt(TH) / np.sqrt(a)

    # window starts: tokens whose gaussian reaches into chunk j
    starts = np.zeros((B, NJ), np.int32)
    span_max = 1
    for b in range(B):
        lo_r, hi_r = c[b] - reach[b], c[b] + reach[b]
        for j in range(NJ):
            qual = np.nonzero((lo_r <= CW * j + CW - 1) & (hi_r >= CW * j))[0]
            if len(qual):
                span_max = max(span_max, int(np.ceil((qual[-1] - qual[0] + 1) / KW)))
                starts[b, j] = qual[0]
            else:
                starts[b, j] = 0
    nkc = span_max
    starts = np.minimum(starts, T_IN - KW * nkc)
    # coverage assert (windows are contiguous token ranges)
    for b in range(B):
        lo_r, hi_r = c[b] - reach[b], c[b] + reach[b]
        for j in range(NJ):
            qual = np.nonzero((lo_r <= CW * j + CW - 1) & (hi_r >= CW * j))[0]
            if len(qual):
                assert starts[b, j] <= qual[0] and qual[-1] < starts[b, j] + KW * nkc

    # gathered per-window tensors
    ea = np.ones((B, T_IN, 257), BF16)
    ea[:, :, :256] = emb.astype(BF16)
    kidx = starts[:, None, :, None] + (
        np.arange(nkc)[None, :, None, None] * KW + np.arange(KW)[None, None, None, :]
    )  # (B, nkc, NJ, KW)
    bidx = np.arange(B)[:, None, None, None]
    eg = ea[bidx, kidx]                       # (B, nkc, NJ, KW, 257)
    eg = eg.transpose(0, 3, 1, 2, 4).copy()   # (B, KW, nkc, NJ, 257)
    cg = c[bidx, kidx]                        # (B, nkc, NJ, KW)
    ag = a[bidx, kidx]
    jgrid = np.arange(NJ, dtype=np.float32)[None, None, :, None] * CW
    sqa_g = np.sqrt(ag)
    sqa = sqa_g.transpose(3, 0, 1, 2).astype(np.float32)                 # (KW,B,nkc,NJ)
    nsqac = (sqa_g * (jgrid - cg)).transpose(3, 0, 1, 2).astype(np.float32)

    # constants: permuted iota (col f of W is frame t' = 4*(f%128) + f//128)
    f = np.arange(CW)
    tperm = (4 * (f % 128) + f // 128).astype(np.float32)
    iota = np.broadcast_to(tperm, (128, CW)).copy()
    # PE in the same permuted layout: tile[q, j*1024 + sub*256 + d] = PE[512j+4q+sub, d]
    pe = _pe_table().reshape(NJ, 128, 4, D).transpose(1, 0, 2, 3).reshape(128, -1)
    pe = pe.astype(BF16)

    key = nkc
    if key not in _CACHE:
        _CACHE[key] = _build(nkc)
    nc = _CACHE[key]

    NP = BL * nkc * NJ
    in_maps = []
    for i in range(N_CORES):
        bs = slice(i * BL, (i + 1) * BL)
        par = np.concatenate(
            [
                sqa[:, bs].reshape(128, NP),
                nsqac[:, bs].reshape(128, NP),
                iota,
            ],
            axis=1,
        ).astype(np.float32)
        in_maps.append({
            "eg": np.ascontiguousarray(eg[bs]),
            "par": np.ascontiguousarray(par),
            "pe": pe,
        })

    res = run_bass_kernel_spmd(nc, in_maps, core_ids=list(range(N_CORES)))
    out = np.concatenate([r["out"] for r in res.results], axis=0)
    return out.astype(np.float32)
